# revision 31
# baseline (speedup 1.0000x reference)
"""Multi-head attention (B=2, S=2048, D=1024, H=16, dk=dv=64) on 8 TRN2 cores.

Sharding: core c -> batch b = c % 2, head-group g = c // 2 (heads 4g..4g+3).
Each core computes its 4 heads' attention for one batch plus the partial
output projection; the host sums the 4 partials per batch and adds the bias.

All matmuls run in 16-bit (fp16 for the Q/K/score path and projections,
bf16 where exp magnitudes require the exponent range): 1 cycle/row on the
PE at full clock and half the HBM/SBUF traffic of the fp32r baseline.

Bias algebra (exact, moved off-device):
  - bk adds only t-independent terms to scores, which cancel in the
    softmax over t -> dropped entirely.
  - bv's contribution is (sum_t softmax) * bv = SCALE * bv per head after
    normalization -> folded into the host-side output bias as
    bo + concat_h(SCALE * bv_h) @ Wo.
  - bq survives (bq . KW[t] varies over t) -> fused into the Q-projection
    PSUM eviction as an ACT bias.

Per-core device pipeline:
  1. KWT [128(=2 heads x dk), S] fp16 pair-stacked projections; QWT same
     with the bq bias; VW projected in natural [t, v] orientation
     (stationary = vt d-chunk tile, moving = wv) straight into the
     [128t, tt, 4*(dv+1)] bf16 tile -- no PE transposes. The 65th column
     per head is a memset ones column (softmax denominator).
  2. scoresT[t, s] = KWT.T @ QWT per head, two heads concurrently via
     64x128 PE row tiling; exp fused into the PSUM->SBUF eviction (ACT),
     output bf16. No max-subtraction (|scores| < 45, exp fits bf16).
  3. ctxT[dv+1, s] = VW1.T @ exp_scoresT accumulated over t; row dv is the
     softmax denominator. Normalize: K=1 matmul broadcasts the denominator
     row to 64 partitions, DVE reciprocal, DVE multiply -> fp16 ctx.
  4. out[s, D] fp16 partial = ctx_allT.T @ Wo, DVE-evicted, DMA'd out.
"""
import os
import sys

sys.path.insert(0, "/opt/trn_rl_repo")
os.environ.setdefault("JAX_PLATFORMS", "axon,cpu")

from contextlib import ExitStack

import numpy as np

import concourse.bacc as bacc
import concourse.tile as tile
from concourse import mybir
from concourse.bass_utils import run_bass_kernel_spmd

FP32 = mybir.dt.float32
FP16 = mybir.dt.float16
BF16 = mybir.dt.bfloat16

B, S, D = 2, 2048, 1024
H, DK, DV = 16, 64, 64
N_CORES = 8
HPC = H // (N_CORES // B)  # heads per core = 4
P = 128
SBLK = 512                # s-block (free dim of scores matmuls)
NBLK = S // SBLK          # 4
NTT = S // P              # 16 t-tiles
NDC = D // P              # 8 contraction chunks
NV = HPC * (DV + 1)       # 260
SCALE = 1.0 / (DK * 2.0)  # folded into Wv (device) and the bv fold (host)


def _build_nc():
    nc = bacc.Bacc("TRN2", target_bir_lowering=False, debug=False,
                   num_devices=N_CORES)
    d = {}
    # all inputs are host-repacked so each SBUF partition's data is one
    # contiguous DRAM run (large DMA packets instead of 1KB fragments)
    for name, shape, dt in [
        ("qt", [NBLK, P, NDC * SBLK], FP16), ("kt", [NBLK, P, NDC * SBLK], FP16),
        ("vt", [NBLK, P, NDC * SBLK], FP16),
        ("wq", [P, NDC, 2 * P], FP16), ("wk", [P, NDC, 2 * P], FP16),
        ("wv", [P, NDC, 2 * P], FP16),
        ("bq2", [P, 2], FP32),
        ("ident", [P, P], FP16),
        ("wo", [P, 2, D], FP16),
    ]:
        d[name] = nc.dram_tensor(name, shape, dt, kind="ExternalInput").ap()
    out_d = nc.dram_tensor("out", [S, D], FP16, kind="ExternalOutput").ap()

    with tile.TileContext(nc) as tc, ExitStack() as ctx:
        const = ctx.enter_context(tc.tile_pool(name="const", bufs=1))
        wpool = ctx.enter_context(tc.tile_pool(name="wpool", bufs=1))
        xtp = ctx.enter_context(tc.tile_pool(name="xtp", bufs=2))
        projp = ctx.enter_context(tc.tile_pool(name="projp", bufs=1))
        expp = ctx.enter_context(tc.tile_pool(name="expp", bufs=1))
        ctxp = ctx.enter_context(tc.tile_pool(name="ctxp", bufs=1))
        outp = ctx.enter_context(tc.tile_pool(name="outp", bufs=2))
        smallp = ctx.enter_context(tc.tile_pool(name="smallp", bufs=2))
        psum = ctx.enter_context(tc.tile_pool(name="psum", bufs=1, space="PSUM"))

        # ---- constants / weights on the ACT DMA ring (inputs stream on the
        # SP and GPSIMD rings concurrently; three rings avoid head-of-line
        # blocking during the DMA-bound projection phase) ----
        wk_sb = wpool.tile([P, NDC, 2 * P], FP16)
        nc.scalar.dma_start(wk_sb[:], d["wk"])
        bq2 = const.tile([P, 2], FP32)
        nc.scalar.dma_start(bq2[:], d["bq2"])
        # rank-1 broadcast ones: hp0 reads row 0, hp1 reads row 64 so the two
        # K=1 matmuls land on disjoint PE row-tiles and run 2-wide
        ones_bf = const.tile([P, DV], BF16)
        nc.vector.memset(ones_bf[:], 1.0)
        ones_f16 = const.tile([P, SBLK], FP16)
        nc.vector.memset(ones_f16[:], 1.0)
        ident_f = const.tile([P, P], FP16)
        nc.scalar.dma_start(ident_f[:], d["ident"])
        wv_sb = wpool.tile([P, NDC, 2 * P], FP16)
        wq_sb = wpool.tile([P, NDC, 2 * P], FP16)
        wo_sb = wpool.tile([P, 2, D], FP16)

        def load_w(sb, name):
            nc.scalar.dma_start(sb[:], d[name])

        # ---- persistent activation tiles ----
        qwt = [projp.tile([P, S], FP16, tag=f"qwt{p_}", name=f"qwt{p_}") for p_ in range(2)]
        kwt = [projp.tile([P, S], FP16, tag=f"kwt{p_}", name=f"kwt{p_}") for p_ in range(2)]
        vwt = [projp.tile([P, S], FP16, tag=f"vwt{p_}", name=f"vwt{p_}") for p_ in range(2)]
        vw = projp.tile([P, NTT, NV], BF16, tag="vw")
        # softmax-denominator ones column (memset once, strided over the
        # 65-wide head slots)
        for hh in range(HPC):
            nc.vector.memset(vw[:, :, hh * (DV + 1) + DV], 1.0)
        ctx_t = [ctxp.tile([P, S], FP16, tag=f"ctx{p_}", name=f"ctx{p_}") for p_ in range(2)]

        def load_chunk(name, ci, tag="xtk", bufs=2, split=False):
            xt = xtp.tile([P, NDC, SBLK], FP16, tag=tag, name="xt", bufs=bufs)
            src = d[name][ci].rearrange("p (dc s) -> p dc s", s=SBLK)
            if split:
                # halve arrival latency: two rings, and the dc<4 matmuls can
                # start as soon as the first half lands
                nc.sync.dma_start(xt[:, 0:NDC // 2, :], src[:, 0:NDC // 2, :])
                nc.gpsimd.dma_start(xt[:, NDC // 2:, :], src[:, NDC // 2:, :])
            else:
                eng = nc.gpsimd if name == "vt" else nc.sync
                eng.dma_start(xt[:], src)
            return xt

        def proj_qk_pair(xt, w_sb, dst, ci, pair, bias=False, tag=None):
            """Project one head-pair of a chunk into dst[pair][:, ci*SBLK:...]."""
            pq = psum.tile([P, SBLK], FP32, tag=tag or ("pj" if pair == 0 else "po"),
                           name="pq")
            for dc in range(NDC):
                nc.tensor.matmul(pq[:], lhsT=w_sb[:, dc, pair * P:(pair + 1) * P],
                                 rhs=xt[:, dc, :], start=(dc == 0), stop=(dc == NDC - 1))
            dst_ap = dst[pair][:, ci * SBLK:(ci + 1) * SBLK]
            if bias:
                nc.vector.scalar_tensor_tensor(
                    dst_ap, pq[:], bq2[:, pair:pair + 1], ones_f16[:],
                    mybir.AluOpType.add, mybir.AluOpType.mult)
            else:
                nc.scalar.copy(dst_ap, pq[:])

        def proj_v(xt, ci):
            """VWT (pair-stacked 512-col streams, full PE rate), then
            PE-transpose 128x128 blocks into vw natural; the DVE eviction
            converts to bf16."""
            for pair in range(2):
                proj_qk_pair(xt, wv_sb, vwt, ci, pair)
            for pair in range(2):
                for c in range(SBLK // P):
                    tt = ci * (SBLK // P) + c
                    tp = psum.tile([P, P], FP16, tag="ct0" if (pair * 4 + c) % 2 == 0 else "ct1",
                                   name="tp")
                    nc.tensor.transpose(
                        tp[:], vwt[pair][:, ci * SBLK + c * P:ci * SBLK + (c + 1) * P],
                        ident_f[:])
                    nc.vector.tensor_copy(
                        vw[:, tt, :].rearrange("p (h v) -> p h v", v=DV + 1)[:, 2 * pair:2 * pair + 2, 0:DV],
                        tp[:].rearrange("p (h v) -> p h v", h=2))

        def attn_alloc(pair):
            return [psum.tile([DV + 1, SBLK], FP32, tag=f"ct{hp}", name=f"ct{hp}")
                    for hp in range(2)]

        def attn_block(pair, b, ct, fillers):
            """Per-2-t-tile pipeline: scores(k) -> exp(k) -> ctx(k), ctx chasing
            exp by one step. One 4-bank scores PSUM per step holds both heads'
            2 t-tiles, evicted by a single FD=2048 exp. `fillers` is a list of
            no-arg callables emitting extra PE work, drained one per step."""
            NK = NTT // 2
            exs = {}
            for k in range(NK + 2):
                if k < NK:
                    sc = [psum.tile([P, 2 * SBLK], FP32, tag=f"sc{hp}", name=f"sc{hp}")
                          for hp in range(2)]
                    for sub in range(2):
                        tt = k * 2 + sub
                        for hp in range(2):
                            lo, hi = hp * DK, (hp + 1) * DK
                            nc.tensor.matmul(
                                sc[hp][:, sub * SBLK:(sub + 1) * SBLK],
                                lhsT=kwt[pair][lo:hi, tt * P:(tt + 1) * P],
                                rhs=qwt[pair][lo:hi, b * SBLK:(b + 1) * SBLK],
                                start=True, stop=True)
                    ex = [expp.tile([P, 2, SBLK], BF16, tag=f"exp{hp}", name=f"exp{hp}", bufs=3)
                          for hp in range(2)]
                    for hp in range(2):
                        nc.scalar.activation(
                            ex[hp][:], sc[hp][:].rearrange("p (u q) -> p u q", u=2),
                            mybir.ActivationFunctionType.Exp)
                    exs[k] = ex
                if fillers:
                    fillers.pop(0)()
                # ctx trails exp by 2 steps: the block's first ctx matmul waits
                # for the ct-psum slot freed by the PREVIOUS block's normalize,
                # so give that chain two steps of slack.
                kc = k - 2
                if kc >= 0:
                    ex = exs.pop(kc)
                    for sub in range(2):
                        tt = kc * 2 + sub
                        for hp in range(2):
                            hh = 2 * pair + hp
                            nc.tensor.matmul(
                                ct[hp][:], lhsT=vw[:, tt, hh * (DV + 1):(hh + 1) * (DV + 1)],
                                rhs=ex[hp][:, sub, :],
                                start=(tt == 0), stop=(tt == NTT - 1))

        def attn_normalize(pair, b, ct):
            # ctx = ct[0:64] * (1 / ct[64]) row-broadcast; the two heads'
            # K=1 broadcast matmuls sit on PE rows 0 / 64 and banks pj / po,
            # so they dispatch 2-wide.
            den = smallp.tile([P, SBLK], BF16, tag="den")
            rb, rcp = [], []
            for hp in range(2):
                nc.vector.tensor_copy(den[hp * DV:hp * DV + 1, :], ct[hp][DV:DV + 1, :])
            for hp in range(2):
                r = psum.tile([DV, SBLK], FP32, tag="pj" if hp == 0 else "po",
                              name="rb")
                nc.tensor.matmul(r[:], lhsT=ones_bf[hp * DV:hp * DV + 1, 0:DV],
                                 rhs=den[hp * DV:hp * DV + 1, :], start=True, stop=True)
                rb.append(r)
            for hp in range(2):
                rc = smallp.tile([DV, SBLK], FP32, tag=f"rcp{hp}")
                nc.vector.reciprocal_approx_fast(rc[:], rb[hp][:])
                rcp.append(rc)
            for hp in range(2):
                nc.vector.tensor_mul(
                    ctx_t[pair][hp * DV:(hp + 1) * DV, b * SBLK:(b + 1) * SBLK],
                    ct[hp][0:DV, :], rcp[hp][:])

        def out_proj_nh(b, st, nh, ob_holder, tag="po"):
            """Half of one 128-row output stripe; nh=1 flushes the full
            [P, D] tile in a single DMA (2KB contiguous rows)."""
            off = b * SBLK + st * P
            po = psum.tile([P, SBLK], FP32, tag=tag, name="po")
            for jc in range(2):
                nc.tensor.matmul(po[:],
                                 lhsT=ctx_t[jc][:, off:off + P],
                                 rhs=wo_sb[:, jc, nh * SBLK:(nh + 1) * SBLK],
                                 start=(jc == 0), stop=(jc == 1))
            if nh == 0:
                ob_holder[st] = outp.tile([P, D], FP16, tag="ob", name="ob")
            ob = ob_holder[st]
            nc.vector.tensor_copy(ob[:, nh * SBLK:(nh + 1) * SBLK], po[:])
            if nh == 1:
                nc.sync.dma_start(out_d[off:off + P, :], ob[:])

        def proj_qk_piece(xt, w_sb, dst, ci, pair, dc_range, pq_holder):
            if dc_range[0] == 0:
                pq_holder[pair] = psum.tile([P, SBLK], FP32, tag="pj", name="pq")
            pq = pq_holder[pair]
            for dc in dc_range:
                nc.tensor.matmul(pq[:], lhsT=w_sb[:, dc, pair * P:(pair + 1) * P],
                                 rhs=xt[:, dc, :], start=(dc == 0), stop=(dc == NDC - 1))
            if dc_range[-1] == NDC - 1:
                nc.vector.scalar_tensor_tensor(
                    dst[pair][:, ci * SBLK:(ci + 1) * SBLK], pq[:],
                    bq2[:, pair:pair + 1], ones_f16[:],
                    mybir.AluOpType.add, mybir.AluOpType.mult)

        # ---- emission schedule ----
        # K and V fully first (attention needs full-T KWT/VW); Q chunk-by-chunk.
        # The next chunk's Q projection and the previous block's output
        # projection are drained into attention's per-step PE slack.
        vts = {}
        for ci in range(NBLK):
            kt = load_chunk("kt", ci)
            if ci == 0:
                load_w(wv_sb, "wv")
                vts[0] = load_chunk("vt", 0, tag="xtv", bufs=2)
                vts[1] = load_chunk("vt", 1, tag="xtv", bufs=2)
            if ci == 2:
                load_w(wq_sb, "wq")
            proj_qk_pair(kt, wk_sb, kwt, ci, 0)
            proj_qk_pair(kt, wk_sb, kwt, ci, 1)
        for ci in range(NBLK):
            vt = vts.pop(ci) if ci in vts else load_chunk("vt", ci, tag="xtv", bufs=2)
            if ci == 0:
                load_w(wo_sb, "wo")
            proj_v(vt, ci)
        qt = load_chunk("qt", 0)
        proj_qk_pair(qt, wq_sb, qwt, 0, 0, bias=True)
        proj_qk_pair(qt, wq_sb, qwt, 0, 1, bias=True)

        def interleave(a, bl):
            out = []
            for i in range(max(len(a), len(bl))):
                if i < len(a):
                    out.append(a[i])
                if i < len(bl):
                    out.append(bl[i])
            return out

        prev_norm = None  # pair-1 normalize deferred into the next block
        for b in range(NBLK):
            have_next = b + 1 < NBLK
            pp = [[], []]
            if have_next:
                qt = load_chunk("qt", b + 1)
                holder = [None, None]
                for pair in range(2):
                    for dcs in ([0, 1], [2, 3], [4, 5], [6, 7]):
                        pp[pair].append(lambda xt=qt, p=pair, r=tuple(dcs), h=holder:
                                        proj_qk_piece(xt, wq_sb, qwt, b + 1, p, r, h))
            op = [[], []]
            if b > 0:
                obh = {}
                for st in range(4):
                    for nh in range(2):
                        op[st // 2].append(lambda s=st, n=nh, h=obh: out_proj_nh(b - 1, s, n, h))
            fill0 = ([prev_norm] if prev_norm else []) + interleave(pp[0], op[0])
            ct0 = attn_alloc(0)
            attn_block(0, b, ct0, fill0)
            fill1 = [lambda bb=b, c=ct0: attn_normalize(0, bb, c)] + interleave(pp[1], op[1])
            ct1 = attn_alloc(1)
            attn_block(1, b, ct1, fill1)
            prev_norm = (lambda bb=b, c=ct1: attn_normalize(1, bb, c))
        prev_norm()
        obh_f = {}
        for st in range(4):
            for nh in range(2):
                out_proj_nh(NBLK - 1, st, nh, obh_f,
                            tag="po" if (st * 2 + nh) % 2 == 0 else "pj")

    nc.compile()
    return nc


_NC_CACHE = None


def _get_nc():
    global _NC_CACHE
    if _NC_CACHE is None:
        _NC_CACHE = _build_nc()
    return _NC_CACHE


def _pack_x(x):
    """[S, D] -> [NBLK, P, NDC*SBLK] fp16: chunk ci, partition p holds the
    contiguous run (dc, s) of X.T[(dc*P + p), ci*SBLK + s]."""
    xt = x.T.astype(np.float16)                       # [D, S]
    xt = xt.reshape(NDC, P, NBLK, SBLK)               # [dc, p, ci, s]
    return np.ascontiguousarray(xt.transpose(2, 1, 0, 3).reshape(NBLK, P, NDC * SBLK))


def _pack_w(w):
    """[D, M] -> [P, NDC, M] fp16 (partition-contiguous weight chunks)."""
    return np.ascontiguousarray(
        w.astype(np.float16).reshape(NDC, P, -1).transpose(1, 0, 2))


def kernel(Q, K, V, Wq, bq, Wk, bk, Wv, bv, Wo, bo, _trace=False, _trace_kwargs=None):
    nc = _get_nc()
    qt_h = [_pack_x(np.asarray(Q[b])) for b in range(B)]
    kt_h = [_pack_x(np.asarray(K[b])) for b in range(B)]
    vt_h = [_pack_x(np.asarray(V[b])) for b in range(B)]

    in_maps = []
    for c in range(N_CORES):
        b, g = c % B, c // B
        hs = list(range(g * HPC, (g + 1) * HPC))
        wq_p = np.concatenate([Wq[h] for h in hs], axis=1)
        wk_p = np.concatenate([Wk[h] for h in hs], axis=1)
        wv_p = np.concatenate([Wv[h] * SCALE for h in hs], axis=1)
        bq2_p = np.stack([
            np.concatenate([bq[hs[0]], bq[hs[1]]]),
            np.concatenate([bq[hs[2]], bq[hs[3]]]),
        ], axis=1)
        wo_p = Wo[g * HPC * DV:(g + 1) * HPC * DV].astype(np.float16)
        in_maps.append({
            "qt": qt_h[b], "kt": kt_h[b], "vt": vt_h[b],
            "wq": _pack_w(wq_p),
            "wk": _pack_w(wk_p),
            "wv": _pack_w(wv_p),
            "bq2": np.ascontiguousarray(bq2_p.astype(np.float32)),
            "ident": np.eye(P, dtype=np.float16),
            "wo": np.ascontiguousarray(wo_p.reshape(2, P, D).transpose(1, 0, 2)),
        })

    kw = {}
    if _trace:
        kw = dict(trace=True, **(_trace_kwargs or {}))
    res = run_bass_kernel_spmd(nc, in_maps, core_ids=list(range(N_CORES)), **kw)

    out = np.zeros((B, S, D), dtype=np.float32)
    for c in range(N_CORES):
        out[c % B] += res.results[c]["out"].astype(np.float32)
    # host-side exact bias fold: bo + concat_h(SCALE * bv_h) @ Wo
    bo_eff = (np.asarray(bo, dtype=np.float64)
              + (np.asarray(bv, dtype=np.float64).reshape(-1) * SCALE) @ np.asarray(Wo, dtype=np.float64))
    out += bo_eff.astype(np.float32)[None, None, :]
    if _trace:
        return out, res
    return out


# revision 32
# speedup vs baseline: 1.0139x; 1.0139x over previous
"""Multi-head attention (B=2, S=2048, D=1024, H=16, dk=dv=64) on 8 TRN2 cores.

Sharding: core c -> batch b = c % 2, head-group g = c // 2 (heads 4g..4g+3).
Each core computes its 4 heads' attention for one batch plus the partial
output projection; the host sums the 4 partials per batch and adds the bias.

All matmuls run in 16-bit (fp16 for the Q/K/score path and projections,
bf16 where exp magnitudes require the exponent range): 1 cycle/row on the
PE at full clock and half the HBM/SBUF traffic of the fp32r baseline.

Bias algebra (exact, moved off-device):
  - bk adds only t-independent terms to scores, which cancel in the
    softmax over t -> dropped entirely.
  - bv's contribution is (sum_t softmax) * bv = SCALE * bv per head after
    normalization -> folded into the host-side output bias as
    bo + concat_h(SCALE * bv_h) @ Wo.
  - bq survives (bq . KW[t] varies over t) -> fused into the Q-projection
    PSUM eviction as an ACT bias.

Per-core device pipeline:
  1. KWT [128(=2 heads x dk), S] fp16 pair-stacked projections; QWT same
     with the bq bias; VW projected in natural [t, v] orientation
     (stationary = vt d-chunk tile, moving = wv) straight into the
     [128t, tt, 4*(dv+1)] bf16 tile -- no PE transposes. The 65th column
     per head is a memset ones column (softmax denominator).
  2. scoresT[t, s] = KWT.T @ QWT per head, two heads concurrently via
     64x128 PE row tiling; exp fused into the PSUM->SBUF eviction (ACT),
     output bf16. No max-subtraction (|scores| < 45, exp fits bf16).
  3. ctxT[dv+1, s] = VW1.T @ exp_scoresT accumulated over t; row dv is the
     softmax denominator. Normalize: K=1 matmul broadcasts the denominator
     row to 64 partitions, DVE reciprocal, DVE multiply -> fp16 ctx.
  4. out[s, D] fp16 partial = ctx_allT.T @ Wo, DVE-evicted, DMA'd out.
"""
import os
import sys

sys.path.insert(0, "/opt/trn_rl_repo")
os.environ.setdefault("JAX_PLATFORMS", "axon,cpu")

from contextlib import ExitStack

import numpy as np

import concourse.bacc as bacc
import concourse.tile as tile
from concourse import mybir
from concourse.bass_utils import run_bass_kernel_spmd

FP32 = mybir.dt.float32
FP16 = mybir.dt.float16
BF16 = mybir.dt.bfloat16

B, S, D = 2, 2048, 1024
H, DK, DV = 16, 64, 64
N_CORES = 8
HPC = H // (N_CORES // B)  # heads per core = 4
P = 128
SBLK = 512                # s-block (free dim of scores matmuls)
NBLK = S // SBLK          # 4
NTT = S // P              # 16 t-tiles
NDC = D // P              # 8 contraction chunks
NV = HPC * (DV + 1)       # 260
SCALE = 1.0 / (DK * 2.0)  # folded into Wv (device) and the bv fold (host)


def _build_nc():
    nc = bacc.Bacc("TRN2", target_bir_lowering=False, debug=False,
                   num_devices=N_CORES)
    d = {}
    # all inputs are host-repacked so each SBUF partition's data is one
    # contiguous DRAM run (large DMA packets instead of 1KB fragments)
    for name, shape, dt in [
        ("qt", [NBLK, P, NDC * SBLK], FP16), ("kt", [NBLK, P, NDC * SBLK], FP16),
        ("vt", [NBLK, P, NDC * SBLK], FP16),
        ("wq", [P, NDC, 2 * P], FP16), ("wk", [P, NDC, 2 * P], FP16),
        ("wv", [P, NDC, 2 * P], FP16),
        ("bq2", [P, 2], FP32),
        ("ident", [P, P], FP16),
        ("wo", [P, 2, D], FP16),
    ]:
        d[name] = nc.dram_tensor(name, shape, dt, kind="ExternalInput").ap()
    out_d = nc.dram_tensor("out", [S, D], FP16, kind="ExternalOutput").ap()

    with tile.TileContext(nc) as tc, ExitStack() as ctx:
        const = ctx.enter_context(tc.tile_pool(name="const", bufs=1))
        wpool = ctx.enter_context(tc.tile_pool(name="wpool", bufs=1))
        xtp = ctx.enter_context(tc.tile_pool(name="xtp", bufs=2))
        projp = ctx.enter_context(tc.tile_pool(name="projp", bufs=1))
        expp = ctx.enter_context(tc.tile_pool(name="expp", bufs=1))
        ctxp = ctx.enter_context(tc.tile_pool(name="ctxp", bufs=1))
        outp = ctx.enter_context(tc.tile_pool(name="outp", bufs=2))
        smallp = ctx.enter_context(tc.tile_pool(name="smallp", bufs=2))
        psum = ctx.enter_context(tc.tile_pool(name="psum", bufs=1, space="PSUM"))

        # ---- constants / weights on the ACT DMA ring (inputs stream on the
        # SP and GPSIMD rings concurrently; three rings avoid head-of-line
        # blocking during the DMA-bound projection phase) ----
        wk_sb = wpool.tile([P, NDC, 2 * P], FP16)
        nc.scalar.dma_start(wk_sb[:], d["wk"])
        bq2 = const.tile([P, 2], FP32)
        nc.scalar.dma_start(bq2[:], d["bq2"])
        # rank-1 broadcast ones: hp0 reads row 0, hp1 reads row 64 so the two
        # K=1 matmuls land on disjoint PE row-tiles and run 2-wide
        ones_bf = const.tile([P, DV], BF16)
        nc.vector.memset(ones_bf[:], 1.0)
        ones_f16 = const.tile([P, SBLK], FP16)
        nc.vector.memset(ones_f16[:], 1.0)
        ident_f = const.tile([P, P], FP16)
        nc.scalar.dma_start(ident_f[:], d["ident"])
        wv_sb = wpool.tile([P, NDC, 2 * P], FP16)
        wq_sb = wpool.tile([P, NDC, 2 * P], FP16)
        wo_sb = wpool.tile([P, 2, D], FP16)

        def load_w(sb, name):
            nc.scalar.dma_start(sb[:], d[name])

        # ---- persistent activation tiles ----
        qwt = [projp.tile([P, S], FP16, tag=f"qwt{p_}", name=f"qwt{p_}") for p_ in range(2)]
        kwt = [projp.tile([P, S], FP16, tag=f"kwt{p_}", name=f"kwt{p_}") for p_ in range(2)]
        vwt = [projp.tile([P, S], FP16, tag=f"vwt{p_}", name=f"vwt{p_}") for p_ in range(2)]
        vw = projp.tile([P, NTT, NV], BF16, tag="vw")
        # softmax-denominator ones column (memset once, strided over the
        # 65-wide head slots)
        for hh in range(HPC):
            nc.vector.memset(vw[:, :, hh * (DV + 1) + DV], 1.0)
        ctx_t = [ctxp.tile([P, S], FP16, tag=f"ctx{p_}", name=f"ctx{p_}") for p_ in range(2)]

        def load_chunk(name, ci, tag="xtk", bufs=2, split=False):
            xt = xtp.tile([P, NDC, SBLK], FP16, tag=tag, name="xt", bufs=bufs)
            src = d[name][ci].rearrange("p (dc s) -> p dc s", s=SBLK)
            if split:
                # halve arrival latency: two rings, and the dc<4 matmuls can
                # start as soon as the first half lands
                nc.sync.dma_start(xt[:, 0:NDC // 2, :], src[:, 0:NDC // 2, :])
                nc.gpsimd.dma_start(xt[:, NDC // 2:, :], src[:, NDC // 2:, :])
            else:
                eng = nc.gpsimd if name == "vt" else nc.sync
                eng.dma_start(xt[:], src)
            return xt

        def proj_qk_pair(xt, w_sb, dst, ci, pair, bias=False, tag=None):
            """Project one head-pair of a chunk into dst[pair][:, ci*SBLK:...]."""
            pq = psum.tile([P, SBLK], FP32, tag=tag or ("pj" if pair == 0 else "po"),
                           name="pq")
            for dc in range(NDC):
                nc.tensor.matmul(pq[:], lhsT=w_sb[:, dc, pair * P:(pair + 1) * P],
                                 rhs=xt[:, dc, :], start=(dc == 0), stop=(dc == NDC - 1))
            dst_ap = dst[pair][:, ci * SBLK:(ci + 1) * SBLK]
            if bias:
                nc.vector.scalar_tensor_tensor(
                    dst_ap, pq[:], bq2[:, pair:pair + 1], ones_f16[:],
                    mybir.AluOpType.add, mybir.AluOpType.mult)
            else:
                nc.scalar.copy(dst_ap, pq[:])

        def proj_v(xt, ci):
            """VWT (pair-stacked 512-col streams, full PE rate), then
            PE-transpose 128x128 blocks into vw natural; the DVE eviction
            converts to bf16."""
            for pair in range(2):
                proj_qk_pair(xt, wv_sb, vwt, ci, pair)
            for pair in range(2):
                for c in range(SBLK // P):
                    tt = ci * (SBLK // P) + c
                    tp = psum.tile([P, P], FP16, tag="ct0" if (pair * 4 + c) % 2 == 0 else "ct1",
                                   name="tp")
                    nc.tensor.transpose(
                        tp[:], vwt[pair][:, ci * SBLK + c * P:ci * SBLK + (c + 1) * P],
                        ident_f[:])
                    nc.vector.tensor_copy(
                        vw[:, tt, :].rearrange("p (h v) -> p h v", v=DV + 1)[:, 2 * pair:2 * pair + 2, 0:DV],
                        tp[:].rearrange("p (h v) -> p h v", h=2))

        def attn_alloc(pair):
            return [psum.tile([DV + 1, SBLK], FP32, tag=f"ct{hp}", name=f"ct{hp}")
                    for hp in range(2)]

        def attn_block(pair, b, ct, fillers):
            """Per-2-t-tile pipeline: scores(k) -> exp(k) -> ctx(k), ctx chasing
            exp by one step. One 4-bank scores PSUM per step holds both heads'
            2 t-tiles, evicted by a single FD=2048 exp. `fillers` is a list of
            no-arg callables emitting extra PE work, drained one per step."""
            NK = NTT // 2
            exs = {}
            for k in range(NK + 2):
                if k < NK:
                    sc = [psum.tile([P, 2 * SBLK], FP32, tag=f"sc{hp}", name=f"sc{hp}")
                          for hp in range(2)]
                    for sub in range(2):
                        tt = k * 2 + sub
                        for hp in range(2):
                            lo, hi = hp * DK, (hp + 1) * DK
                            nc.tensor.matmul(
                                sc[hp][:, sub * SBLK:(sub + 1) * SBLK],
                                lhsT=kwt[pair][lo:hi, tt * P:(tt + 1) * P],
                                rhs=qwt[pair][lo:hi, b * SBLK:(b + 1) * SBLK],
                                start=True, stop=True)
                    ex = [expp.tile([P, 2, SBLK], BF16, tag=f"exp{hp}", name=f"exp{hp}", bufs=3)
                          for hp in range(2)]
                    for hp in range(2):
                        nc.scalar.activation(
                            ex[hp][:], sc[hp][:].rearrange("p (u q) -> p u q", u=2),
                            mybir.ActivationFunctionType.Exp)
                    exs[k] = ex
                if fillers:
                    fillers.pop(0)()
                # ctx trails exp by 2 steps: the block's first ctx matmul waits
                # for the ct-psum slot freed by the PREVIOUS block's normalize,
                # so give that chain two steps of slack.
                kc = k - 2
                if kc >= 0:
                    ex = exs.pop(kc)
                    for sub in range(2):
                        tt = kc * 2 + sub
                        for hp in range(2):
                            hh = 2 * pair + hp
                            nc.tensor.matmul(
                                ct[hp][:], lhsT=vw[:, tt, hh * (DV + 1):(hh + 1) * (DV + 1)],
                                rhs=ex[hp][:, sub, :],
                                start=(tt == 0), stop=(tt == NTT - 1))

        def attn_normalize(pair, b, ct):
            # ctx = ct[0:64] * (1 / ct[64]) row-broadcast; the two heads'
            # K=1 broadcast matmuls sit on PE rows 0 / 64 and banks pj / po,
            # so they dispatch 2-wide.
            den = smallp.tile([P, SBLK], BF16, tag="den")
            rb, rcp = [], []
            for hp in range(2):
                nc.vector.tensor_copy(den[hp * DV:hp * DV + 1, :], ct[hp][DV:DV + 1, :])
            for hp in range(2):
                r = psum.tile([DV, SBLK], FP32, tag="pj" if hp == 0 else "po",
                              name="rb")
                nc.tensor.matmul(r[:], lhsT=ones_bf[hp * DV:hp * DV + 1, 0:DV],
                                 rhs=den[hp * DV:hp * DV + 1, :], start=True, stop=True)
                rb.append(r)
            for hp in range(2):
                rc = smallp.tile([DV, SBLK], FP32, tag=f"rcp{hp}")
                nc.vector.reciprocal_approx_fast(rc[:], rb[hp][:])
                rcp.append(rc)
            for hp in range(2):
                nc.vector.tensor_mul(
                    ctx_t[pair][hp * DV:(hp + 1) * DV, b * SBLK:(b + 1) * SBLK],
                    ct[hp][0:DV, :], rcp[hp][:])

        def out_proj_nh(b, st, nh, ob_holder, tag="po"):
            """Half of one 128-row output stripe; nh=1 flushes the full
            [P, D] tile in a single DMA (2KB contiguous rows)."""
            off = b * SBLK + st * P
            po = psum.tile([P, SBLK], FP32, tag=tag, name="po")
            for jc in range(2):
                nc.tensor.matmul(po[:],
                                 lhsT=ctx_t[jc][:, off:off + P],
                                 rhs=wo_sb[:, jc, nh * SBLK:(nh + 1) * SBLK],
                                 start=(jc == 0), stop=(jc == 1))
            if nh == 0:
                ob_holder[st] = outp.tile([P, D], FP16, tag="ob", name="ob")
            ob = ob_holder[st]
            nc.vector.tensor_copy(ob[:, nh * SBLK:(nh + 1) * SBLK], po[:])
            if nh == 1:
                nc.sync.dma_start(out_d[off:off + P, :], ob[:])

        def proj_qk_piece(xt, w_sb, dst, ci, pair, dc_range, pq_holder):
            if dc_range[0] == 0:
                pq_holder[pair] = psum.tile([P, SBLK], FP32, tag="pj", name="pq")
            pq = pq_holder[pair]
            for dc in dc_range:
                nc.tensor.matmul(pq[:], lhsT=w_sb[:, dc, pair * P:(pair + 1) * P],
                                 rhs=xt[:, dc, :], start=(dc == 0), stop=(dc == NDC - 1))
            if dc_range[-1] == NDC - 1:
                nc.vector.scalar_tensor_tensor(
                    dst[pair][:, ci * SBLK:(ci + 1) * SBLK], pq[:],
                    bq2[:, pair:pair + 1], ones_f16[:],
                    mybir.AluOpType.add, mybir.AluOpType.mult)

        # ---- emission schedule ----
        # K and V fully first (attention needs full-T KWT/VW); Q chunk-by-chunk.
        # The next chunk's Q projection and the previous block's output
        # projection are drained into attention's per-step PE slack.
        vts = {}
        for ci in range(NBLK):
            # kt1 is the tightest arrival: split it across both input rings,
            # ahead of the vt prefetches on gpsimd
            kt = load_chunk("kt", ci, split=(ci == 1))
            if ci == 1:
                load_w(wv_sb, "wv")
                vts[0] = load_chunk("vt", 0, tag="xtv", bufs=2)
                vts[1] = load_chunk("vt", 1, tag="xtv", bufs=2)
            if ci == 2:
                load_w(wq_sb, "wq")
            proj_qk_pair(kt, wk_sb, kwt, ci, 0)
            proj_qk_pair(kt, wk_sb, kwt, ci, 1)
        for ci in range(NBLK):
            vt = vts.pop(ci) if ci in vts else load_chunk("vt", ci, tag="xtv", bufs=2)
            if ci == 0:
                load_w(wo_sb, "wo")
            proj_v(vt, ci)
        qt = load_chunk("qt", 0)
        proj_qk_pair(qt, wq_sb, qwt, 0, 0, bias=True)
        proj_qk_pair(qt, wq_sb, qwt, 0, 1, bias=True)

        def interleave(a, bl):
            out = []
            for i in range(max(len(a), len(bl))):
                if i < len(a):
                    out.append(a[i])
                if i < len(bl):
                    out.append(bl[i])
            return out

        prev_norm = None  # pair-1 normalize deferred into the next block
        for b in range(NBLK):
            have_next = b + 1 < NBLK
            pp = [[], []]
            if have_next:
                qt = load_chunk("qt", b + 1)
                holder = [None, None]
                for pair in range(2):
                    for dcs in ([0, 1], [2, 3], [4, 5], [6, 7]):
                        pp[pair].append(lambda xt=qt, p=pair, r=tuple(dcs), h=holder:
                                        proj_qk_piece(xt, wq_sb, qwt, b + 1, p, r, h))
            op = [[], []]
            if b > 0:
                obh = {}
                for st in range(4):
                    for nh in range(2):
                        op[st // 2].append(lambda s=st, n=nh, h=obh: out_proj_nh(b - 1, s, n, h))
            fill0 = ([prev_norm] if prev_norm else []) + interleave(pp[0], op[0])
            ct0 = attn_alloc(0)
            attn_block(0, b, ct0, fill0)
            fill1 = [lambda bb=b, c=ct0: attn_normalize(0, bb, c)] + interleave(pp[1], op[1])
            ct1 = attn_alloc(1)
            attn_block(1, b, ct1, fill1)
            prev_norm = (lambda bb=b, c=ct1: attn_normalize(1, bb, c))
        prev_norm()
        obh_f = {}
        for st in range(4):
            for nh in range(2):
                out_proj_nh(NBLK - 1, st, nh, obh_f,
                            tag="po" if (st * 2 + nh) % 2 == 0 else "pj")

    nc.compile()
    return nc


_NC_CACHE = None


def _get_nc():
    global _NC_CACHE
    if _NC_CACHE is None:
        _NC_CACHE = _build_nc()
    return _NC_CACHE


def _pack_x(x):
    """[S, D] -> [NBLK, P, NDC*SBLK] fp16: chunk ci, partition p holds the
    contiguous run (dc, s) of X.T[(dc*P + p), ci*SBLK + s]."""
    xt = x.T.astype(np.float16)                       # [D, S]
    xt = xt.reshape(NDC, P, NBLK, SBLK)               # [dc, p, ci, s]
    return np.ascontiguousarray(xt.transpose(2, 1, 0, 3).reshape(NBLK, P, NDC * SBLK))


def _pack_w(w):
    """[D, M] -> [P, NDC, M] fp16 (partition-contiguous weight chunks)."""
    return np.ascontiguousarray(
        w.astype(np.float16).reshape(NDC, P, -1).transpose(1, 0, 2))


def kernel(Q, K, V, Wq, bq, Wk, bk, Wv, bv, Wo, bo, _trace=False, _trace_kwargs=None):
    nc = _get_nc()
    qt_h = [_pack_x(np.asarray(Q[b])) for b in range(B)]
    kt_h = [_pack_x(np.asarray(K[b])) for b in range(B)]
    vt_h = [_pack_x(np.asarray(V[b])) for b in range(B)]

    in_maps = []
    for c in range(N_CORES):
        b, g = c % B, c // B
        hs = list(range(g * HPC, (g + 1) * HPC))
        wq_p = np.concatenate([Wq[h] for h in hs], axis=1)
        wk_p = np.concatenate([Wk[h] for h in hs], axis=1)
        wv_p = np.concatenate([Wv[h] * SCALE for h in hs], axis=1)
        bq2_p = np.stack([
            np.concatenate([bq[hs[0]], bq[hs[1]]]),
            np.concatenate([bq[hs[2]], bq[hs[3]]]),
        ], axis=1)
        wo_p = Wo[g * HPC * DV:(g + 1) * HPC * DV].astype(np.float16)
        in_maps.append({
            "qt": qt_h[b], "kt": kt_h[b], "vt": vt_h[b],
            "wq": _pack_w(wq_p),
            "wk": _pack_w(wk_p),
            "wv": _pack_w(wv_p),
            "bq2": np.ascontiguousarray(bq2_p.astype(np.float32)),
            "ident": np.eye(P, dtype=np.float16),
            "wo": np.ascontiguousarray(wo_p.reshape(2, P, D).transpose(1, 0, 2)),
        })

    kw = {}
    if _trace:
        kw = dict(trace=True, **(_trace_kwargs or {}))
    res = run_bass_kernel_spmd(nc, in_maps, core_ids=list(range(N_CORES)), **kw)

    out = np.zeros((B, S, D), dtype=np.float32)
    for c in range(N_CORES):
        out[c % B] += res.results[c]["out"].astype(np.float32)
    # host-side exact bias fold: bo + concat_h(SCALE * bv_h) @ Wo
    bo_eff = (np.asarray(bo, dtype=np.float64)
              + (np.asarray(bv, dtype=np.float64).reshape(-1) * SCALE) @ np.asarray(Wo, dtype=np.float64))
    out += bo_eff.astype(np.float32)[None, None, :]
    if _trace:
        return out, res
    return out


# revision 33
# speedup vs baseline: 1.0231x; 1.0092x over previous
"""Multi-head attention (B=2, S=2048, D=1024, H=16, dk=dv=64) on 8 TRN2 cores.

Sharding: core c -> batch b = c % 2, head-group g = c // 2 (heads 4g..4g+3).
Each core computes its 4 heads' attention for one batch plus the partial
output projection; the host sums the 4 partials per batch and adds the bias.

All matmuls run in 16-bit (fp16 for the Q/K/score path and projections,
bf16 where exp magnitudes require the exponent range): 1 cycle/row on the
PE at full clock and half the HBM/SBUF traffic of the fp32r baseline.

Bias algebra (exact, moved off-device):
  - bk adds only t-independent terms to scores, which cancel in the
    softmax over t -> dropped entirely.
  - bv's contribution is (sum_t softmax) * bv = SCALE * bv per head after
    normalization -> folded into the host-side output bias as
    bo + concat_h(SCALE * bv_h) @ Wo.
  - bq survives (bq . KW[t] varies over t) -> fused into the Q-projection
    PSUM eviction as an ACT bias.

Per-core device pipeline:
  1. KWT/VWT [128(=2 heads x dk), S] fp16 pair-stacked projections (512-col
     streams, full PE rate; 256-col "natural" V-projection measured 2.2x
     slower per column from exposed per-instruction overhead); QWT adds the
     bq bias via a DVE scalar_tensor_tensor eviction. VWT is PE-transposed
     128x128-block-wise into the [128t, tt, 4*(dv+1)] bf16 vw tile; the
     65th column per head is a memset ones column (softmax denominator).
  2. scoresT[t, s] = KWT.T @ QWT per head, two heads concurrently via
     64x128 PE row tiling; exp fused into the PSUM->SBUF eviction (ACT),
     output bf16. No max-subtraction (|scores| < 45, exp fits bf16).
  3. ctxT[dv+1, s] = VW1.T @ exp_scoresT accumulated over t; row dv is the
     softmax denominator. Normalize: K=1 matmul broadcasts the denominator
     row to 64 partitions, DVE reciprocal, DVE multiply -> fp16 ctx.
  4. out[s, D] fp16 partial = ctx_allT.T @ Wo, DVE-evicted, DMA'd out.
"""
import os
import sys

sys.path.insert(0, "/opt/trn_rl_repo")
os.environ.setdefault("JAX_PLATFORMS", "axon,cpu")

from contextlib import ExitStack

import numpy as np

import concourse.bacc as bacc
import concourse.tile as tile
from concourse import mybir
from concourse.bass_utils import run_bass_kernel_spmd

FP32 = mybir.dt.float32
FP16 = mybir.dt.float16
BF16 = mybir.dt.bfloat16

B, S, D = 2, 2048, 1024
H, DK, DV = 16, 64, 64
N_CORES = 8
HPC = H // (N_CORES // B)  # heads per core = 4
P = 128
SBLK = 512                # s-block (free dim of scores matmuls)
NBLK = S // SBLK          # 4
NTT = S // P              # 16 t-tiles
NDC = D // P              # 8 contraction chunks
NV = HPC * (DV + 1)       # 260
SCALE = 1.0 / (DK * 2.0)  # folded into Wv (device) and the bv fold (host)


def _build_nc():
    nc = bacc.Bacc("TRN2", target_bir_lowering=False, debug=False,
                   num_devices=N_CORES)
    d = {}
    # all inputs are host-repacked so each SBUF partition's data is one
    # contiguous DRAM run (large DMA packets instead of 1KB fragments)
    for name, shape, dt in [
        ("qt", [NBLK, P, NDC * SBLK], FP16), ("kt", [NBLK, P, NDC * SBLK], FP16),
        ("vt", [NBLK, P, NDC * SBLK], FP16),
        ("wq", [P, NDC, 2 * P], FP16), ("wk", [P, NDC, 2 * P], FP16),
        ("wv", [P, NDC, 2 * P], FP16),
        ("bq2", [P, 2], FP32),
        ("ident", [P, P], FP16),
        ("wo", [P, 2, D], FP16),
    ]:
        d[name] = nc.dram_tensor(name, shape, dt, kind="ExternalInput").ap()
    out_d = nc.dram_tensor("out", [S, D], FP16, kind="ExternalOutput").ap()

    with tile.TileContext(nc) as tc, ExitStack() as ctx:
        const = ctx.enter_context(tc.tile_pool(name="const", bufs=1))
        wpool = ctx.enter_context(tc.tile_pool(name="wpool", bufs=1))
        xtp = ctx.enter_context(tc.tile_pool(name="xtp", bufs=2))
        projp = ctx.enter_context(tc.tile_pool(name="projp", bufs=1))
        expp = ctx.enter_context(tc.tile_pool(name="expp", bufs=1))
        ctxp = ctx.enter_context(tc.tile_pool(name="ctxp", bufs=1))
        outp = ctx.enter_context(tc.tile_pool(name="outp", bufs=2))
        smallp = ctx.enter_context(tc.tile_pool(name="smallp", bufs=2))
        psum = ctx.enter_context(tc.tile_pool(name="psum", bufs=1, space="PSUM"))

        # ---- constants / weights on the ACT DMA ring (inputs stream on the
        # SP and GPSIMD rings concurrently; three rings avoid head-of-line
        # blocking during the DMA-bound projection phase) ----
        wk_sb = wpool.tile([P, NDC, 2 * P], FP16)
        nc.scalar.dma_start(wk_sb[:], d["wk"])
        bq2 = const.tile([P, 2], FP32)
        nc.scalar.dma_start(bq2[:], d["bq2"])
        # rank-1 broadcast ones: hp0 reads row 0, hp1 reads row 64 so the two
        # K=1 matmuls land on disjoint PE row-tiles and run 2-wide
        ones_bf = const.tile([P, DV], BF16)
        nc.vector.memset(ones_bf[:], 1.0)
        ones_f16 = const.tile([P, SBLK], FP16)
        nc.vector.memset(ones_f16[:], 1.0)
        ident_f = const.tile([P, P], FP16)
        nc.scalar.dma_start(ident_f[:], d["ident"])
        wv_sb = wpool.tile([P, NDC, 2 * P], FP16)
        wq_sb = wpool.tile([P, NDC, 2 * P], FP16)
        wo_sb = wpool.tile([P, 2, D], FP16)

        def load_w(sb, name):
            nc.scalar.dma_start(sb[:], d[name])

        # ---- persistent activation tiles ----
        qwt = [projp.tile([P, S], FP16, tag=f"qwt{p_}", name=f"qwt{p_}") for p_ in range(2)]
        kwt = [projp.tile([P, S], FP16, tag=f"kwt{p_}", name=f"kwt{p_}") for p_ in range(2)]
        vwt = [projp.tile([P, S], FP16, tag=f"vwt{p_}", name=f"vwt{p_}") for p_ in range(2)]
        vw = projp.tile([P, NTT, NV], BF16, tag="vw")
        # softmax-denominator ones column (memset once, strided over the
        # 65-wide head slots)
        for hh in range(HPC):
            nc.vector.memset(vw[:, :, hh * (DV + 1) + DV], 1.0)
        ctx_t = [ctxp.tile([P, S], FP16, tag=f"ctx{p_}", name=f"ctx{p_}") for p_ in range(2)]

        def load_chunk(name, ci, tag="xtk", bufs=2, split=False):
            xt = xtp.tile([P, NDC, SBLK], FP16, tag=tag, name="xt", bufs=bufs)
            src = d[name][ci].rearrange("p (dc s) -> p dc s", s=SBLK)
            if split:
                # halve arrival latency: two rings, and the dc<4 matmuls can
                # start as soon as the first half lands
                nc.sync.dma_start(xt[:, 0:NDC // 2, :], src[:, 0:NDC // 2, :])
                nc.gpsimd.dma_start(xt[:, NDC // 2:, :], src[:, NDC // 2:, :])
            else:
                eng = nc.gpsimd if name == "vt" else nc.sync
                eng.dma_start(xt[:], src)
            return xt

        def proj_qk_pair(xt, w_sb, dst, ci, pair, bias=False, tag=None):
            """Project one head-pair of a chunk into dst[pair][:, ci*SBLK:...]."""
            pq = psum.tile([P, SBLK], FP32, tag=tag or ("pj" if pair == 0 else "po"),
                           name="pq")
            for dc in range(NDC):
                nc.tensor.matmul(pq[:], lhsT=w_sb[:, dc, pair * P:(pair + 1) * P],
                                 rhs=xt[:, dc, :], start=(dc == 0), stop=(dc == NDC - 1))
            dst_ap = dst[pair][:, ci * SBLK:(ci + 1) * SBLK]
            if bias:
                nc.vector.scalar_tensor_tensor(
                    dst_ap, pq[:], bq2[:, pair:pair + 1], ones_f16[:],
                    mybir.AluOpType.add, mybir.AluOpType.mult)
            else:
                nc.scalar.copy(dst_ap, pq[:])

        def proj_v(xt, ci):
            """VWT (pair-stacked 512-col streams, full PE rate), then
            PE-transpose 128x128 blocks into vw natural; the DVE eviction
            converts to bf16."""
            for pair in range(2):
                proj_qk_pair(xt, wv_sb, vwt, ci, pair)
            for pair in range(2):
                for c in range(SBLK // P):
                    tt = ci * (SBLK // P) + c
                    tp = psum.tile([P, P], FP16, tag="ct0" if (pair * 4 + c) % 2 == 0 else "ct1",
                                   name="tp")
                    nc.tensor.transpose(
                        tp[:], vwt[pair][:, ci * SBLK + c * P:ci * SBLK + (c + 1) * P],
                        ident_f[:])
                    nc.vector.tensor_copy(
                        vw[:, tt, :].rearrange("p (h v) -> p h v", v=DV + 1)[:, 2 * pair:2 * pair + 2, 0:DV],
                        tp[:].rearrange("p (h v) -> p h v", h=2))

        def attn_alloc(pair):
            return [psum.tile([DV + 1, SBLK], FP32, tag=f"ct{hp}", name=f"ct{hp}")
                    for hp in range(2)]

        def attn_block(pair, b, ct, fillers):
            """Per-2-t-tile pipeline: scores(k) -> exp(k) -> ctx(k), ctx chasing
            exp by one step. One 4-bank scores PSUM per step holds both heads'
            2 t-tiles, evicted by a single FD=2048 exp. `fillers` is a list of
            no-arg callables emitting extra PE work, drained one per step."""
            NK = NTT // 2
            exs = {}
            for k in range(NK + 2):
                if k < NK:
                    sc = [psum.tile([P, 2 * SBLK], FP32, tag=f"sc{hp}", name=f"sc{hp}")
                          for hp in range(2)]
                    for sub in range(2):
                        tt = k * 2 + sub
                        for hp in range(2):
                            lo, hi = hp * DK, (hp + 1) * DK
                            nc.tensor.matmul(
                                sc[hp][:, sub * SBLK:(sub + 1) * SBLK],
                                lhsT=kwt[pair][lo:hi, tt * P:(tt + 1) * P],
                                rhs=qwt[pair][lo:hi, b * SBLK:(b + 1) * SBLK],
                                start=True, stop=True)
                    ex = [expp.tile([P, 2, SBLK], BF16, tag=f"exp{hp}", name=f"exp{hp}", bufs=3)
                          for hp in range(2)]
                    for hp in range(2):
                        nc.scalar.activation(
                            ex[hp][:], sc[hp][:].rearrange("p (u q) -> p u q", u=2),
                            mybir.ActivationFunctionType.Exp)
                    exs[k] = ex
                if fillers:
                    fillers.pop(0)()
                # ctx trails exp by 2 steps: the block's first ctx matmul waits
                # for the ct-psum slot freed by the PREVIOUS block's normalize,
                # so give that chain two steps of slack.
                kc = k - 2
                if kc >= 0:
                    ex = exs.pop(kc)
                    for sub in range(2):
                        tt = kc * 2 + sub
                        for hp in range(2):
                            hh = 2 * pair + hp
                            nc.tensor.matmul(
                                ct[hp][:], lhsT=vw[:, tt, hh * (DV + 1):(hh + 1) * (DV + 1)],
                                rhs=ex[hp][:, sub, :],
                                start=(tt == 0), stop=(tt == NTT - 1))

        def attn_normalize(pair, b, ct):
            # ctx = ct[0:64] * (1 / ct[64]) row-broadcast; the two heads'
            # K=1 broadcast matmuls sit on PE rows 0 / 64 and banks pj / po,
            # so they dispatch 2-wide.
            den = smallp.tile([P, SBLK], BF16, tag="den")
            rb, rcp = [], []
            for hp in range(2):
                nc.vector.tensor_copy(den[hp * DV:hp * DV + 1, :], ct[hp][DV:DV + 1, :])
            for hp in range(2):
                r = psum.tile([DV, SBLK], FP32, tag="pj" if hp == 0 else "po",
                              name="rb")
                nc.tensor.matmul(r[:], lhsT=ones_bf[hp * DV:hp * DV + 1, 0:DV],
                                 rhs=den[hp * DV:hp * DV + 1, :], start=True, stop=True)
                rb.append(r)
            for hp in range(2):
                rc = smallp.tile([DV, SBLK], FP32, tag=f"rcp{hp}")
                nc.vector.reciprocal_approx_fast(rc[:], rb[hp][:])
                rcp.append(rc)
            for hp in range(2):
                nc.vector.tensor_mul(
                    ctx_t[pair][hp * DV:(hp + 1) * DV, b * SBLK:(b + 1) * SBLK],
                    ct[hp][0:DV, :], rcp[hp][:])

        def out_proj_nh(b, st, nh, ob_holder, tag="po"):
            """Half of one 128-row output stripe; nh=1 flushes the full
            [P, D] tile in a single DMA (2KB contiguous rows)."""
            off = b * SBLK + st * P
            po = psum.tile([P, SBLK], FP32, tag=tag, name="po")
            for jc in range(2):
                nc.tensor.matmul(po[:],
                                 lhsT=ctx_t[jc][:, off:off + P],
                                 rhs=wo_sb[:, jc, nh * SBLK:(nh + 1) * SBLK],
                                 start=(jc == 0), stop=(jc == 1))
            if nh == 0:
                ob_holder[st] = outp.tile([P, D], FP16, tag="ob", name="ob")
            ob = ob_holder[st]
            nc.vector.tensor_copy(ob[:, nh * SBLK:(nh + 1) * SBLK], po[:])
            if nh == 1:
                nc.sync.dma_start(out_d[off:off + P, :], ob[:])

        def proj_qk_piece(xt, w_sb, dst, ci, pair, dc_range, pq_holder):
            if dc_range[0] == 0:
                pq_holder[pair] = psum.tile([P, SBLK], FP32, tag="pj", name="pq")
            pq = pq_holder[pair]
            for dc in dc_range:
                nc.tensor.matmul(pq[:], lhsT=w_sb[:, dc, pair * P:(pair + 1) * P],
                                 rhs=xt[:, dc, :], start=(dc == 0), stop=(dc == NDC - 1))
            if dc_range[-1] == NDC - 1:
                nc.vector.scalar_tensor_tensor(
                    dst[pair][:, ci * SBLK:(ci + 1) * SBLK], pq[:],
                    bq2[:, pair:pair + 1], ones_f16[:],
                    mybir.AluOpType.add, mybir.AluOpType.mult)

        # ---- emission schedule ----
        # K and V fully first (attention needs full-T KWT/VW); Q chunk-by-chunk.
        # The next chunk's Q projection and the previous block's output
        # projection are drained into attention's per-step PE slack.
        vts = {}
        for ci in range(NBLK):
            # kt1 is the tightest arrival: split it across both input rings,
            # ahead of the vt prefetches on gpsimd
            kt = load_chunk("kt", ci, split=(ci == 1))
            if ci == 1:
                load_w(wv_sb, "wv")
                vts[0] = load_chunk("vt", 0, tag="xtv", bufs=2)
                vts[1] = load_chunk("vt", 1, tag="xtv", bufs=2)
            if ci == 2:
                load_w(wq_sb, "wq")
            proj_qk_pair(kt, wk_sb, kwt, ci, 0)
            proj_qk_pair(kt, wk_sb, kwt, ci, 1)
        for ci in range(NBLK):
            vt = vts.pop(ci) if ci in vts else load_chunk("vt", ci, tag="xtv", bufs=2)
            if ci == 0:
                load_w(wo_sb, "wo")
            proj_v(vt, ci)
        qt = load_chunk("qt", 0)
        proj_qk_pair(qt, wq_sb, qwt, 0, 0, bias=True)
        proj_qk_pair(qt, wq_sb, qwt, 0, 1, bias=True)

        def interleave(a, bl):
            out = []
            for i in range(max(len(a), len(bl))):
                if i < len(a):
                    out.append(a[i])
                if i < len(bl):
                    out.append(bl[i])
            return out

        prev_norm = None  # pair-1 normalize deferred into the next block
        for b in range(NBLK):
            have_next = b + 1 < NBLK
            pp = [[], []]
            if have_next:
                qt = load_chunk("qt", b + 1)
                holder = [None, None]
                for pair in range(2):
                    for dcs in ([0, 1], [2, 3], [4, 5], [6, 7]):
                        pp[pair].append(lambda xt=qt, p=pair, r=tuple(dcs), h=holder:
                                        proj_qk_piece(xt, wq_sb, qwt, b + 1, p, r, h))
            op = [[], []]
            if b > 0:
                obh = {}
                for st in range(4):
                    for nh in range(2):
                        op[st // 2].append(lambda s=st, n=nh, h=obh: out_proj_nh(b - 1, s, n, h))
            fill0 = ([prev_norm] if prev_norm else []) + interleave(pp[0], op[0])
            ct0 = attn_alloc(0)
            attn_block(0, b, ct0, fill0)
            fill1 = [lambda bb=b, c=ct0: attn_normalize(0, bb, c)] + interleave(pp[1], op[1])
            ct1 = attn_alloc(1)
            attn_block(1, b, ct1, fill1)
            prev_norm = (lambda bb=b, c=ct1: attn_normalize(1, bb, c))
        prev_norm()
        obh_f = {}
        for st in range(4):
            for nh in range(2):
                out_proj_nh(NBLK - 1, st, nh, obh_f,
                            tag="po" if (st * 2 + nh) % 2 == 0 else "pj")

    nc.compile()
    return nc


_NC_CACHE = None


def _get_nc():
    global _NC_CACHE
    if _NC_CACHE is None:
        _NC_CACHE = _build_nc()
    return _NC_CACHE


def _pack_x(x):
    """[S, D] -> [NBLK, P, NDC*SBLK] fp16: chunk ci, partition p holds the
    contiguous run (dc, s) of X.T[(dc*P + p), ci*SBLK + s]."""
    xt = x.T.astype(np.float16)                       # [D, S]
    xt = xt.reshape(NDC, P, NBLK, SBLK)               # [dc, p, ci, s]
    return np.ascontiguousarray(xt.transpose(2, 1, 0, 3).reshape(NBLK, P, NDC * SBLK))


def _pack_w(w):
    """[D, M] -> [P, NDC, M] fp16 (partition-contiguous weight chunks)."""
    return np.ascontiguousarray(
        w.astype(np.float16).reshape(NDC, P, -1).transpose(1, 0, 2))


def kernel(Q, K, V, Wq, bq, Wk, bk, Wv, bv, Wo, bo, _trace=False, _trace_kwargs=None):
    nc = _get_nc()
    qt_h = [_pack_x(np.asarray(Q[b])) for b in range(B)]
    kt_h = [_pack_x(np.asarray(K[b])) for b in range(B)]
    vt_h = [_pack_x(np.asarray(V[b])) for b in range(B)]

    in_maps = []
    for c in range(N_CORES):
        b, g = c % B, c // B
        hs = list(range(g * HPC, (g + 1) * HPC))
        wq_p = np.concatenate([Wq[h] for h in hs], axis=1)
        wk_p = np.concatenate([Wk[h] for h in hs], axis=1)
        wv_p = np.concatenate([Wv[h] * SCALE for h in hs], axis=1)
        bq2_p = np.stack([
            np.concatenate([bq[hs[0]], bq[hs[1]]]),
            np.concatenate([bq[hs[2]], bq[hs[3]]]),
        ], axis=1)
        wo_p = Wo[g * HPC * DV:(g + 1) * HPC * DV].astype(np.float16)
        in_maps.append({
            "qt": qt_h[b], "kt": kt_h[b], "vt": vt_h[b],
            "wq": _pack_w(wq_p),
            "wk": _pack_w(wk_p),
            "wv": _pack_w(wv_p),
            "bq2": np.ascontiguousarray(bq2_p.astype(np.float32)),
            "ident": np.eye(P, dtype=np.float16),
            "wo": np.ascontiguousarray(wo_p.reshape(2, P, D).transpose(1, 0, 2)),
        })

    kw = {}
    if _trace:
        kw = dict(trace=True, **(_trace_kwargs or {}))
    res = run_bass_kernel_spmd(nc, in_maps, core_ids=list(range(N_CORES)), **kw)

    out = np.zeros((B, S, D), dtype=np.float32)
    for c in range(N_CORES):
        out[c % B] += res.results[c]["out"].astype(np.float32)
    # host-side exact bias fold: bo + concat_h(SCALE * bv_h) @ Wo
    bo_eff = (np.asarray(bo, dtype=np.float64)
              + (np.asarray(bv, dtype=np.float64).reshape(-1) * SCALE) @ np.asarray(Wo, dtype=np.float64))
    out += bo_eff.astype(np.float32)[None, None, :]
    if _trace:
        return out, res
    return out
